# revision 2
# baseline (speedup 1.0000x reference)
"""EnhancedCrossAttention on 8 Trainium2 NeuronCores.

Sharding: core c = 4*b + g handles batch b and head-group g (4 of 16 heads).
Wq/Wk/Wv split column-wise per head group, Wo row-wise; partial outputs
summed on host (tensor-parallel allreduce done at gather time).

v2: bf16 end-to-end on the matmul path (halves DMA bytes and SBUF), coarse
whole-chunk DMAs (cuts per-DMA HWDGE/SEQ serialization ~8x), obj
pre-transposed on host, softmax denominator divide (no reciprocal in the
tail chain), last-chunk head order 1,0,3,2 so the Y start-group (kt=0)
unblocks first.

Per-core device kernel:
  P1: QpT[256,1024] = (Wq_g*scale)^T-projection (transposed layout, bf16)
  P2: per kv-chunk(512): KpT[256,512] proj (transposed), Vp[512,260] proj
      (natural, 4 heads x (64 cols + ones col))
  P3: per chunk/head/kv-tile: S^T[128kv,1024q] = K Q^T in PSUM; P^T =
      exp(obj*S^T) via ACT per-partition scale, bf16 out; O^T[65,1024] +=
      [V|1]^T P^T in PSUM per chunk, DVE-added into SBUF across chunks
      (row 64 = softmax denominator l)
  P4: per head: PE ones-outer-product broadcast of l -> DVE divide;
      Y[1024,1024] = O^T.T @ Wo_g -> bf16 out.
"""

import numpy as np

DIM = 1024
H = 16
HD = 64
B = 2
NQ = 1024
NKV = 4096
HPG = 4           # heads per group (per core)
DH = HPG * HD     # 256 head-dim columns per core
NCORES = 8
KV_CHUNK = 512
N_CHUNKS = NKV // KV_CHUNK
KT = DIM // 128   # k-tiles over DIM

_prog_cache = {}


def _build(has_bq, has_bk, has_bv):
    key = (has_bq, has_bk, has_bv)
    if key in _prog_cache:
        return _prog_cache[key]

    import concourse.mybir as mybir
    import concourse.tile as tile
    from concourse import bacc

    f32 = mybir.dt.float32
    f32r = mybir.dt.float32r
    bf16 = mybir.dt.bfloat16
    EXP = mybir.ActivationFunctionType.Exp
    DIV = mybir.AluOpType.divide

    nc = bacc.Bacc("TRN2")
    xqt = nc.dram_tensor("xqt", [DIM, NQ], bf16, kind="ExternalInput")
    xkt = nc.dram_tensor("xkt", [DIM, NKV], bf16, kind="ExternalInput")
    xvt = nc.dram_tensor("xvt", [DIM, NKV], bf16, kind="ExternalInput")
    wq = nc.dram_tensor("wq", [DIM, DH], bf16, kind="ExternalInput")
    wk = nc.dram_tensor("wk", [DIM, DH], bf16, kind="ExternalInput")
    wv = nc.dram_tensor("wv", [DIM, DH], bf16, kind="ExternalInput")
    wo = nc.dram_tensor("wo", [DH, DIM], bf16, kind="ExternalInput")
    objt = nc.dram_tensor("objt", [128, NKV // 128], f32, kind="ExternalInput")
    bq = nc.dram_tensor("bq", [DH], f32, kind="ExternalInput") if has_bq else None
    bk = nc.dram_tensor("bk", [DH], f32, kind="ExternalInput") if has_bk else None
    bv = nc.dram_tensor("bv", [DH], bf16, kind="ExternalInput") if has_bv else None
    y = nc.dram_tensor("y", [NQ, DIM], bf16, kind="ExternalOutput")

    with tile.TileContext(nc) as tc:
        with tc.tile_pool(name="const", bufs=1) as cpool:
            wq_sb = cpool.tile([128, KT, DH], bf16, tag="wq")
            wk_sb = cpool.tile([128, KT, DH], bf16, tag="wk")
            wv_sb = cpool.tile([128, KT, DH], bf16, tag="wv")
            wo_sb = cpool.tile([128, DH // 128, DIM], bf16, tag="wo")
            obj_sb = cpool.tile([128, NKV // 128], f32, tag="obj")
            ones_sb = cpool.tile([128, 128], f32r, tag="ones")
            qpt = cpool.tile([128, 2, NQ], bf16, tag="qpt")
            ot = cpool.tile([128, 2, NQ], bf16, tag="ot")
            oacc = [
                cpool.tile([65, NQ], f32r, tag=f"oacc{h}", name=f"oacc{h}")
                for h in range(HPG)
            ]

            # DMA order = startup critical path: wq, xq feed P1; wk+xk(c0)
            # feed the first K-projection; everything else in first-use order.
            nc.sync.dma_start(wq_sb[:], wq[:].rearrange("(k p) n -> p k n", p=128))
            nc.gpsimd.memset(ones_sb[:].bitcast(f32), 1.0)
            if has_bq:
                bq_sb = cpool.tile([128, 2], f32, tag="bq")
                nc.sync.dma_start(bq_sb[:], bq[:].rearrange("(m p) -> p m", p=128))
            if has_bk:
                bk_sb = cpool.tile([128, 2], f32, tag="bk")
                nc.sync.dma_start(bk_sb[:], bk[:].rearrange("(m p) -> p m", p=128))
            if has_bv:
                ones_bf = cpool.tile([1, 128], bf16, tag="onesbf")
                nc.gpsimd.memset(ones_bf[:], 1.0)
                bv_sb = cpool.tile([1, DH], bf16, tag="bv")
                nc.sync.dma_start(bv_sb[:], bv[:].rearrange("(a n) -> a n", a=1))

            with (
                tc.tile_pool(name="pj", bufs=2, space="PSUM") as pjpool,
                tc.tile_pool(name="spsum", bufs=2, space="PSUM") as spool,
                tc.tile_pool(name="opsum", bufs=1, space="PSUM") as opool,
            ):
                # ---- P1: Q projection (transposed out) ----
                with tc.tile_pool(name="xq", bufs=1) as xqpool:
                    xq_sb = xqpool.tile([128, KT, NQ], bf16, tag="xq")
                    nc.sync.dma_start(
                        xq_sb[:], xqt[:].rearrange("(k p) n -> p k n", p=128)
                    )
                    nc.sync.dma_start(
                        wk_sb[:], wk[:].rearrange("(k p) n -> p k n", p=128)
                    )
                    for m in range(2):
                        for n in range(2):
                            ps = pjpool.tile([128, 512], f32, tag="pj")
                            for k in range(KT):
                                nc.tensor.matmul(
                                    ps[:],
                                    wq_sb[:, k, m * 128 : (m + 1) * 128],
                                    xq_sb[:, k, n * 512 : (n + 1) * 512],
                                    start=(k == 0),
                                    stop=(k == KT - 1),
                                )
                            dst = qpt[:, m, n * 512 : (n + 1) * 512]
                            if has_bq:
                                nc.vector.tensor_scalar_add(
                                    dst, ps[:], bq_sb[:, m : m + 1]
                                )
                            else:
                                nc.vector.tensor_copy(dst, ps[:])

                # ---- P2+P3: stream kv chunks; project K/V; attention ----
                with (
                    tc.tile_pool(name="xk", bufs=2) as xkpool,
                    tc.tile_pool(name="xv", bufs=2) as xvpool,
                    tc.tile_pool(name="kpt", bufs=2) as kptpool,
                    tc.tile_pool(name="vp", bufs=2) as vppool,
                    tc.tile_pool(name="pt", bufs=4) as ptpool,
                    tc.tile_pool(name="norm", bufs=2) as npool,
                ):
                    xk_re = xkt[:].rearrange("(k p) n -> p k n", p=128)
                    xv_re = xvt[:].rearrange("(k p) n -> p k n", p=128)

                    def load_k(c):
                        cs = slice(c * KV_CHUNK, (c + 1) * KV_CHUNK)
                        xk_c = xkpool.tile(
                            [128, KT, KV_CHUNK], bf16, tag="xk", name="xk_c"
                        )
                        nc.sync.dma_start(xk_c[:], xk_re[:, :, cs])
                        return xk_c

                    def load_v(c):
                        cs = slice(c * KV_CHUNK, (c + 1) * KV_CHUNK)
                        xv_c = xvpool.tile(
                            [128, KT, KV_CHUNK], bf16, tag="xv", name="xv_c"
                        )
                        nc.sync.dma_start(xv_c[:], xv_re[:, :, cs])
                        return xv_c

                    def normalize_head(h):
                        # PE broadcast of l (row 64 of oacc) over 64
                        # partitions, then DVE divide -> normalized O_h^T
                        m = h // 2
                        otmp = None
                        if h % 2 == 1:
                            otmp = npool.tile([64, NQ], bf16, tag="otmp", name="otmp")
                        for n in range(2):
                            nsl = slice(n * 512, (n + 1) * 512)
                            psr = pjpool.tile([128, 512], f32, tag="pj", name="psr")
                            nc.tensor.matmul(
                                psr[0:64, :],
                                ones_sb[64:65, 0:64],
                                oacc[h][64:65, nsl],
                                start=True,
                                stop=True,
                            )
                            dst = otmp[:, nsl] if h % 2 else ot[0:64, m, nsl]
                            with nc.allow_low_precision("softmax normalize"):
                                nc.vector.tensor_tensor(
                                    dst, oacc[h][0:64, nsl], psr[0:64, :], DIV
                                )
                        if h % 2 == 1:
                            nc.sync.dma_start(ot[64:128, m, :], otmp[:, :])

                    prefetched = {0: (load_k(0), load_v(0))}
                    nc.sync.dma_start(
                        wv_sb[:], wv[:].rearrange("(k p) n -> p k n", p=128)
                    )
                    nc.sync.dma_start(obj_sb[:], objt[:])
                    ps_o = {}
                    for c in range(N_CHUNKS):
                        if c in prefetched:
                            xk_c, xv_c = prefetched.pop(c)
                        else:
                            xk_c, xv_c = prefetched.pop(c, None) or (None, None)
                        if xk_c is None:
                            xk_c, xv_c = load_k(c), load_v(c)
                        if c + 1 < N_CHUNKS:
                            prefetched[c + 1] = (load_k(c + 1), load_v(c + 1))
                        if c == 5:
                            # wo for P4: load in the tail of the streaming
                            # phase when DMA has spare bandwidth
                            nc.sync.dma_start(
                                wo_sb[:],
                                wo[:].rearrange("(t p) n -> p t n", p=128),
                            )
                        # K^T projection for this chunk
                        kpt_c = kptpool.tile([128, 2, KV_CHUNK], bf16, tag="kpt")
                        for m in range(2):
                            ps = pjpool.tile([128, 512], f32, tag="pj")
                            for k in range(KT):
                                nc.tensor.matmul(
                                    ps[:],
                                    wk_sb[:, k, m * 128 : (m + 1) * 128],
                                    xk_c[:, k, :],
                                    start=(k == 0),
                                    stop=(k == KT - 1),
                                )
                            if has_bk:
                                nc.vector.tensor_scalar_add(
                                    kpt_c[:, m, :], ps[:], bk_sb[:, m : m + 1]
                                )
                            else:
                                nc.vector.tensor_copy(kpt_c[:, m, :], ps[:])
                        # V projection (natural layout + ones columns)
                        vp_c = vppool.tile([128, 4, HPG * 65], bf16, tag="vp")
                        nc.gpsimd.memset(vp_c[:], 1.0)
                        for t in range(4):
                            ps = pjpool.tile([128, 512], f32, tag="pj")
                            psv = ps[:, 0:DH]
                            for k in range(KT):
                                nc.tensor.matmul(
                                    psv,
                                    xv_c[:, k, t * 128 : (t + 1) * 128],
                                    wv_sb[:, k, :],
                                    start=(k == 0),
                                    stop=(k == KT - 1 and not has_bv),
                                )
                            if has_bv:
                                nc.tensor.matmul(
                                    psv,
                                    ones_bf[0:1, 0:128],
                                    bv_sb[0:1, :],
                                    start=False,
                                    stop=True,
                                )
                            nc.vector.tensor_copy(
                                vp_c[:, t, :].rearrange("p (h e) -> p h e", h=HPG)[
                                    :, :, 0:HD
                                ],
                                psv.rearrange("p (h e) -> p h e", h=HPG),
                            )
                        # attention on this chunk; in the last chunk run
                        # heads 1,0,3,2: kt=0's heads (0,1) normalize first
                        # (they feed the Y start-group), and each pair ends
                        # on an even head (no cross-partition DMA hop)
                        h_order = (
                            [1, 0, 3, 2] if c == N_CHUNKS - 1 else range(HPG)
                        )
                        for h in h_order:
                            hb = (h % 2) * 64
                            m = h // 2
                            ps_o[h] = opool.tile(
                                [65, NQ], f32, tag="o", name=f"ps_o{h}"
                            )
                            for t in range(4):
                                ps_s = spool.tile([128, NQ], f32, tag="s")
                                for n in range(2):
                                    nc.tensor.matmul(
                                        ps_s[:, n * 512 : (n + 1) * 512],
                                        kpt_c[hb : hb + 64, m, t * 128 : (t + 1) * 128],
                                        qpt[hb : hb + 64, m, n * 512 : (n + 1) * 512],
                                        start=True,
                                        stop=True,
                                    )
                                pt_t = ptpool.tile([128, NQ], bf16, tag="pt")
                                ti = c * 4 + t
                                nc.scalar.activation(
                                    pt_t[:], ps_s[:], EXP,
                                    scale=obj_sb[:, ti : ti + 1],
                                )
                                for n in range(2):
                                    nc.tensor.matmul(
                                        ps_o[h][:, n * 512 : (n + 1) * 512],
                                        vp_c[:, t, h * 65 : (h + 1) * 65],
                                        pt_t[:, n * 512 : (n + 1) * 512],
                                        start=(t == 0),
                                        stop=(t == 3),
                                    )
                            if c == 0:
                                nc.vector.tensor_copy(oacc[h][:], ps_o[h][:])
                            else:
                                nc.vector.tensor_add(
                                    oacc[h][:], oacc[h][:], ps_o[h][:]
                                )
                            if c == N_CHUNKS - 1:
                                normalize_head(h)

                    # ---- P4b: Y = O^T.T @ Wo (reuses s psum) ----
                    with tc.tile_pool(name="yb", bufs=3) as ypool:
                        for mq in range(NQ // 128):
                            psy = spool.tile([128, NQ], f32, tag="s", name="psy")
                            for kt2 in (0, 1):
                                for n in range(2):
                                    nc.tensor.matmul(
                                        psy[:, n * 512 : (n + 1) * 512],
                                        ot[:, kt2, mq * 128 : (mq + 1) * 128],
                                        wo_sb[:, kt2, n * 512 : (n + 1) * 512],
                                        start=(kt2 == 0),
                                        stop=(kt2 == 1),
                                    )
                            yt = ypool.tile([128, NQ], bf16, tag="yt")
                            nc.scalar.copy(yt[:], psy[:])
                            nc.sync.dma_start(
                                y[mq * 128 : (mq + 1) * 128, :], yt[:]
                            )

    nc.compile()
    _prog_cache[key] = nc
    return nc


def kernel(query, key, value, objectness_scores, Wq, bq, Wk, bk, Wv, bv, Wo, bo,
           _trace=False):
    import ml_dtypes
    from concourse.bass_utils import run_bass_kernel_spmd

    f = np.float32
    bft = ml_dtypes.bfloat16
    query = np.asarray(query, f)
    key_ = np.asarray(key, f)
    value = np.asarray(value, f)
    objs = np.asarray(objectness_scores, f)
    Wq = np.asarray(Wq, f); bq = np.asarray(bq, f)
    Wk = np.asarray(Wk, f); bk = np.asarray(bk, f)
    Wv = np.asarray(Wv, f); bv = np.asarray(bv, f)
    Wo = np.asarray(Wo, f); bo = np.asarray(bo, f)

    scale = np.float32(HD ** -0.5)
    has_bq = bool(np.any(bq)); has_bk = bool(np.any(bk)); has_bv = bool(np.any(bv))
    nc = _build(has_bq, has_bk, has_bv)

    # per-batch transposed inputs in bf16 (shared across the 4 cores of b)
    xqt_b = [np.ascontiguousarray(query[b].T.astype(bft)) for b in range(B)]
    xkt_b = [np.ascontiguousarray(key_[b].T.astype(bft)) for b in range(B)]
    xvt_b = [np.ascontiguousarray(value[b].T.astype(bft)) for b in range(B)]
    objt_b = [np.ascontiguousarray(objs[b].reshape(-1, 128).T) for b in range(B)]

    in_maps = []
    for c in range(NCORES):
        b, g = divmod(c, NCORES // B)
        sl = slice(g * DH, (g + 1) * DH)
        m = {
            "xqt": xqt_b[b],
            "xkt": xkt_b[b],
            "xvt": xvt_b[b],
            "wq": np.ascontiguousarray((Wq[:, sl] * scale).astype(bft)),
            "wk": np.ascontiguousarray(Wk[:, sl].astype(bft)),
            "wv": np.ascontiguousarray(Wv[:, sl].astype(bft)),
            "wo": np.ascontiguousarray(Wo[sl, :].astype(bft)),
            "objt": objt_b[b],
        }
        if has_bq:
            m["bq"] = np.ascontiguousarray(bq[sl] * scale)
        if has_bk:
            m["bk"] = np.ascontiguousarray(bk[sl])
        if has_bv:
            m["bv"] = np.ascontiguousarray(bv[sl].astype(bft))
        in_maps.append(m)

    res = run_bass_kernel_spmd(
        nc, in_maps, core_ids=list(range(NCORES)), trace=_trace
    )
    out = np.zeros((B, NQ, DIM), np.float64)
    for c in range(NCORES):
        out[c // (NCORES // B)] += np.asarray(res.results[c]["y"]).astype(np.float64)
    out += bo.astype(np.float64)
    result = out.astype(np.float32)
    if _trace:
        return result, res
    return result


# revision 10
# speedup vs baseline: 1.1404x; 1.1404x over previous
"""EnhancedCrossAttention on 8 Trainium2 NeuronCores.

Sharding: core c = 4*b + g handles batch b and head-group g (4 of 16 heads).
Wq/Wk/Wv split column-wise per head group, Wo row-wise; partial outputs
summed on host (tensor-parallel allreduce done at gather time).

v4: hybrid dtypes + pipelined startup/tail. Projection matmuls (Q/K/V) run
bf16<->bf16 (halves the dominant DMA streams; their stationary operands are
DMA-fed well in advance so the per-matmul Ldweights never stalls the PE).
The latency-sensitive attention matmuls (S, PV, Y, normalize broadcast)
stay f32r (self-loading: no Ldweights, so PSUM-copy -> stationary chains
can't drop the PE p-state). Startup interleaves wq/xq/wk/xk0/xv0 DMA
slices per k so the PE chases the DMA stream with no serial phase. Per
head, all four S tiles are emitted (and exp'd) before the four PV tiles,
which gives the PE ~3.4us of S work to hide the cross-chunk oacc DVE add.
Softmax normalize uses DVE divide; the last chunk runs heads 1,0,3,2 so
Y's kt=0 groups unblock first; Y runs in its own 4-buffer PSUM pool with
mq quads interleaved (kt=0 x4 then kt=1 x4) to absorb the final
normalize chain.

Per-core device kernel:
  P1: QpT[256,1024] = (Wq_g*scale)^T-projection (transposed layout)
  P2: per kv-chunk(512): KpT[256,512] proj (transposed), Vp[512,260] proj
      (natural, 4 heads x (64 cols + ones col))
  P3: per chunk/head/kv-tile: S^T[128kv,1024q] = K Q^T; P^T = exp(obj*S^T)
      (obj rides the ACT per-partition scale); O^T[65,1024] += [V|1]^T P^T
      accumulated in PSUM per chunk, DVE-added into SBUF across chunks
      (row 64 = softmax denominator l)
  P4: per head: PE ones-outer-product broadcast of l -> DVE divide;
      Y[1024,1024] = O^T.T @ Wo_g -> bf16 out.
"""

import numpy as np

DIM = 1024
H = 16
HD = 64
B = 2
NQ = 1024
NKV = 4096
HPG = 4           # heads per group (per core)
DH = HPG * HD     # 256 head-dim columns per core
NCORES = 8
KV_CHUNK = 512
N_CHUNKS = NKV // KV_CHUNK
KT = DIM // 128   # k-tiles over DIM

_prog_cache = {}


def _build(has_bq, has_bk, has_bv):
    key = (has_bq, has_bk, has_bv)
    if key in _prog_cache:
        return _prog_cache[key]

    import concourse.mybir as mybir
    import concourse.tile as tile
    from concourse import bacc

    f32 = mybir.dt.float32
    f32r = mybir.dt.float32r
    bf16 = mybir.dt.bfloat16
    EXP = mybir.ActivationFunctionType.Exp
    MULT = mybir.AluOpType.mult

    nc = bacc.Bacc("TRN2")
    xqt = nc.dram_tensor("xqt", [DIM, NQ], bf16, kind="ExternalInput")
    xkt = nc.dram_tensor("xkt", [DIM, NKV], bf16, kind="ExternalInput")
    xvt = nc.dram_tensor("xvt", [DIM, NKV], bf16, kind="ExternalInput")
    wq = nc.dram_tensor("wq", [DIM, DH], bf16, kind="ExternalInput")
    wk = nc.dram_tensor("wk", [DIM, DH], bf16, kind="ExternalInput")
    wv = nc.dram_tensor("wv", [DIM, DH], bf16, kind="ExternalInput")
    wo = nc.dram_tensor("wo", [DH, DIM], f32r, kind="ExternalInput")
    objt = nc.dram_tensor("objt", [128, NKV // 128], f32, kind="ExternalInput")
    bq = nc.dram_tensor("bq", [DH], f32, kind="ExternalInput") if has_bq else None
    bk = nc.dram_tensor("bk", [DH], f32, kind="ExternalInput") if has_bk else None
    bv = nc.dram_tensor("bv", [DH], bf16, kind="ExternalInput") if has_bv else None
    y = nc.dram_tensor("y", [NQ, DIM], bf16, kind="ExternalOutput")

    with tile.TileContext(nc) as tc:
        with tc.tile_pool(name="const", bufs=1) as cpool:
            wq_sb = cpool.tile([128, KT, DH], bf16, tag="wq")
            wk_sb = cpool.tile([128, KT, DH], bf16, tag="wk")
            wv_sb = cpool.tile([128, KT, DH], bf16, tag="wv")
            wo_sb = cpool.tile([128, DH // 128, DIM], f32r, tag="wo")
            obj_sb = cpool.tile([128, NKV // 128], f32, tag="obj")
            ones_sb = cpool.tile([128, 128], f32r, tag="ones")
            qpt = cpool.tile([128, 2, NQ], f32r, tag="qpt")
            ot = cpool.tile([128, 2, NQ], f32r, tag="ot")
            oacc = [
                cpool.tile([65, NQ], f32r, tag=f"oacc{h}", name=f"oacc{h}")
                for h in range(HPG)
            ]

            nc.gpsimd.memset(ones_sb[:].bitcast(f32), 1.0)
            if has_bq:
                bq_sb = cpool.tile([128, 2], f32, tag="bq")
                nc.sync.dma_start(bq_sb[:], bq[:].rearrange("(m p) -> p m", p=128))
            if has_bk:
                bk_sb = cpool.tile([128, 2], f32, tag="bk")
                nc.sync.dma_start(bk_sb[:], bk[:].rearrange("(m p) -> p m", p=128))
            if has_bv:
                ones_bf = cpool.tile([1, 128], bf16, tag="onesbf")
                nc.gpsimd.memset(ones_bf[:], 1.0)
                bv_sb = cpool.tile([1, DH], bf16, tag="bv")
                nc.sync.dma_start(bv_sb[:], bv[:].rearrange("(a n) -> a n", a=1))

            wq_re = wq[:].rearrange("(k p) n -> p k n", p=128)
            wk_re = wk[:].rearrange("(k p) n -> p k n", p=128)
            xq_re = xqt[:].rearrange("(k p) n -> p k n", p=128)
            xk_re = xkt[:].rearrange("(k p) n -> p k n", p=128)
            xv_re = xvt[:].rearrange("(k p) n -> p k n", p=128)

            with (
                tc.tile_pool(name="xk", bufs=2) as xkpool,
                tc.tile_pool(name="xv", bufs=2) as xvpool,
                tc.tile_pool(name="kpt", bufs=2) as kptpool,
                tc.tile_pool(name="vp", bufs=2) as vppool,
                tc.tile_pool(name="pt", bufs=6) as ptpool,
                tc.tile_pool(name="norm", bufs=2) as npool,
            ):
                xk0 = xkpool.tile([128, KT, KV_CHUNK], bf16, tag="xk", name="xk_c")
                xv0 = xvpool.tile([128, KT, KV_CHUNK], bf16, tag="xv", name="xv_c")
                cs0 = slice(0, KV_CHUNK)
                kpt0 = kptpool.tile([128, 2, KV_CHUNK], f32r, tag="kpt")

                # ---- P1: Q projection; startup DMA chase ----
                # DMA streams in PE need-order at ~1.8us granularity (the
                # ~0.9us DMA-completion sem latency makes finer slicing a
                # loss): wq/xq k-quarters with wk + xk0-halves woven in.
                # The PE chases: Qproj quarters with K-proj(c0) interleaved
                # where its data lands. K-proj(c0) shares the startup PSUM
                # pool, so the chunk loop skips c0's K-proj.
                with (
                    tc.tile_pool(name="qpj", bufs=6, space="PSUM") as qpjpool,
                    tc.tile_pool(name="xq", bufs=1) as xqpool,
                ):
                    xq_sb = xqpool.tile([128, KT, NQ], bf16, tag="xq")
                    for k in (0, 1):
                        nc.sync.dma_start(wq_sb[:, k, :], wq_re[:, k, :])
                        nc.sync.dma_start(xq_sb[:, k, :], xq_re[:, k, :])
                    for k in (2, 3):
                        nc.sync.dma_start(wq_sb[:, k, :], wq_re[:, k, :])
                        nc.sync.dma_start(xq_sb[:, k, :], xq_re[:, k, :])
                    nc.sync.dma_start(wk_sb[:], wk_re[:])
                    nc.sync.dma_start(xk0[:, 0:4, :], xk_re[:, 0:4, cs0])
                    for k in (4, 5):
                        nc.sync.dma_start(wq_sb[:, k, :], wq_re[:, k, :])
                        nc.sync.dma_start(xq_sb[:, k, :], xq_re[:, k, :])
                    nc.sync.dma_start(xk0[:, 4:8, :], xk_re[:, 4:8, cs0])
                    for k in (6, 7):
                        nc.sync.dma_start(wq_sb[:, k, :], wq_re[:, k, :])
                        nc.sync.dma_start(xq_sb[:, k, :], xq_re[:, k, :])
                    nc.sync.dma_start(xv0[:, 0:4, :], xv_re[:, 0:4, cs0])
                    nc.sync.dma_start(xv0[:, 4:8, :], xv_re[:, 4:8, cs0])
                    nc.sync.dma_start(obj_sb[:], objt[:])
                    nc.sync.dma_start(
                        wv_sb[:], wv[:].rearrange("(k p) n -> p k n", p=128)
                    )

                    qg = [[qpjpool.tile([128, 512], f32, tag="qpj",
                                        name=f"qg{m}{n}") for n in range(2)]
                          for m in range(2)]
                    kg = [qpjpool.tile([128, 512], f32, tag="qpj",
                                       name=f"kg{m}") for m in range(2)]

                    def qproj_ks(ks):
                        for k in ks:
                            for m in range(2):
                                for n in range(2):
                                    nc.tensor.matmul(
                                        qg[m][n][:],
                                        wq_sb[:, k, m * 128 : (m + 1) * 128],
                                        xq_sb[:, k, n * 512 : (n + 1) * 512],
                                        start=(k == 0),
                                        stop=(k == KT - 1),
                                    )

                    def kproj0_ks(m, ks):
                        for k in ks:
                            nc.tensor.matmul(
                                kg[m][:],
                                wk_sb[:, k, m * 128 : (m + 1) * 128],
                                xk0[:, k, :],
                                start=(k == 0),
                                stop=(k == KT - 1),
                            )

                    qproj_ks((0, 1))
                    qproj_ks((2, 3))
                    kproj0_ks(0, (0, 1, 2, 3))
                    qproj_ks((4, 5))
                    kproj0_ks(0, (4, 5, 6, 7))
                    kproj0_ks(1, range(KT))
                    qproj_ks((6, 7))
                    for m in range(2):
                        if has_bk:
                            nc.vector.tensor_scalar_add(
                                kpt0[:, m, :], kg[m][:], bk_sb[:, m : m + 1]
                            )
                        else:
                            nc.vector.tensor_copy(kpt0[:, m, :], kg[m][:])
                        for n in range(2):
                            dst = qpt[:, m, n * 512 : (n + 1) * 512]
                            if has_bq:
                                nc.vector.tensor_scalar_add(
                                    dst, qg[m][n][:], bq_sb[:, m : m + 1]
                                )
                            else:
                                nc.vector.tensor_copy(dst, qg[m][n][:])

                # ---- P2+P3: stream kv chunks; project K/V; attention ----
                with (
                    tc.tile_pool(name="pj", bufs=2, space="PSUM") as pjpool,
                    tc.tile_pool(name="spsum", bufs=2, space="PSUM") as spool,
                    tc.tile_pool(name="opsum", bufs=1, space="PSUM") as opool,
                ):
                    def load_k(c):
                        cs = slice(c * KV_CHUNK, (c + 1) * KV_CHUNK)
                        xk_c = xkpool.tile(
                            [128, KT, KV_CHUNK], bf16, tag="xk", name="xk_c"
                        )
                        nc.sync.dma_start(xk_c[:], xk_re[:, :, cs])
                        return xk_c

                    def load_v(c):
                        cs = slice(c * KV_CHUNK, (c + 1) * KV_CHUNK)
                        xv_c = xvpool.tile(
                            [128, KT, KV_CHUNK], bf16, tag="xv", name="xv_c"
                        )
                        nc.sync.dma_start(xv_c[:], xv_re[:, :, cs])
                        return xv_c

                    def normalize_head(h):
                        # recip of l (row 64 of oacc) -> PE ones-outer-
                        # product broadcast over 64 partitions -> DVE mult
                        m = h // 2
                        rec = npool.tile([65, NQ], f32r, tag="rec", name="rec")
                        with nc.allow_low_precision("softmax recip rounding"):
                            nc.vector.reciprocal(
                                rec[64:65, :], oacc[h][64:65, :]
                            )
                        otmp = None
                        if h % 2 == 1:
                            otmp = npool.tile(
                                [64, NQ], f32r, tag="otmp", name="otmp"
                            )
                        for n in range(2):
                            nsl = slice(n * 512, (n + 1) * 512)
                            psr = pjpool.tile([128, 512], f32, tag="pj", name="psr")
                            nc.tensor.matmul(
                                psr[0:64, :],
                                ones_sb[64:65, 0:64],
                                rec[64:65, nsl],
                                start=True,
                                stop=True,
                            )
                            dst = otmp[:, nsl] if h % 2 else ot[0:64, m, nsl]
                            nc.vector.tensor_tensor(
                                dst, oacc[h][0:64, nsl], psr[0:64, :], MULT
                            )
                        if h % 2 == 1:
                            nc.sync.dma_start(ot[64:128, m, :], otmp[:, :])

                    prefetched = {0: (xk0, xv0)}
                    ps_o = {}
                    for c in range(N_CHUNKS):
                        xk_c, xv_c = prefetched.pop(c, (None, None))
                        if xk_c is None:
                            xk_c, xv_c = load_k(c), load_v(c)
                        if c + 1 < N_CHUNKS:
                            prefetched[c + 1] = (load_k(c + 1), load_v(c + 1))
                        if c == 5:
                            # wo for P4: load in the tail of the streaming
                            # phase when DMA has spare bandwidth
                            nc.sync.dma_start(
                                wo_sb[:],
                                wo[:].rearrange("(t p) n -> p t n", p=128),
                            )
                        # K^T projection for this chunk (bf16 x bf16);
                        # c0's was folded into the startup chase
                        if c == 0:
                            kpt_c = kpt0
                        else:
                            kpt_c = kptpool.tile(
                                [128, 2, KV_CHUNK], f32r, tag="kpt"
                            )
                            for m in range(2):
                                ps = pjpool.tile([128, 512], f32, tag="pj")
                                for k in range(KT):
                                    nc.tensor.matmul(
                                        ps[:],
                                        wk_sb[:, k, m * 128 : (m + 1) * 128],
                                        xk_c[:, k, :],
                                        start=(k == 0),
                                        stop=(k == KT - 1),
                                    )
                                if has_bk:
                                    nc.vector.tensor_scalar_add(
                                        kpt_c[:, m, :], ps[:], bk_sb[:, m : m + 1]
                                    )
                                else:
                                    nc.vector.tensor_copy(kpt_c[:, m, :], ps[:])
                        # V projection (bf16 x bf16; natural + ones cols)
                        vp_c = vppool.tile([128, 4, HPG * 65], f32r, tag="vp")
                        nc.gpsimd.memset(vp_c[:].bitcast(f32), 1.0)
                        for t in range(4):
                            ps = pjpool.tile([128, 512], f32, tag="pj")
                            psv = ps[:, 0:DH]
                            for k in range(KT):
                                nc.tensor.matmul(
                                    psv,
                                    xv_c[:, k, t * 128 : (t + 1) * 128],
                                    wv_sb[:, k, :],
                                    start=(k == 0),
                                    stop=(k == KT - 1 and not has_bv),
                                )
                            if has_bv:
                                nc.tensor.matmul(
                                    psv,
                                    ones_bf[0:1, 0:128],
                                    bv_sb[0:1, :],
                                    start=False,
                                    stop=True,
                                )
                            nc.vector.tensor_copy(
                                vp_c[:, t, :].rearrange(
                                    "p (h e) -> p h e", h=HPG
                                )[:, :, 0:HD],
                                psv.rearrange("p (h e) -> p h e", h=HPG),
                            )
                        # attention; last chunk runs heads 1,0,3,2 so Y's
                        # kt=0 heads normalize first and each pair ends on
                        # an even head (no cross-partition DMA hop)
                        h_order = (
                            [1, 0, 3, 2] if c == N_CHUNKS - 1 else range(HPG)
                        )
                        for h in h_order:
                            hb = (h % 2) * 64
                            m = h // 2
                            ps_o[h] = opool.tile(
                                [65, NQ], f32, tag="o", name=f"ps_o{h}"
                            )
                            pts = []
                            for t in range(4):
                                ps_s = spool.tile([128, NQ], f32, tag="s")
                                for n in range(2):
                                    nc.tensor.matmul(
                                        ps_s[:, n * 512 : (n + 1) * 512],
                                        kpt_c[hb : hb + 64, m,
                                              t * 128 : (t + 1) * 128],
                                        qpt[hb : hb + 64, m,
                                            n * 512 : (n + 1) * 512],
                                        start=True,
                                        stop=True,
                                    )
                                pt_t = ptpool.tile([128, NQ], f32r, tag="pt")
                                ti = c * 4 + t
                                nc.scalar.activation(
                                    pt_t[:], ps_s[:], EXP,
                                    scale=obj_sb[:, ti : ti + 1],
                                )
                                pts.append(pt_t)
                            for t in range(4):
                                for n in range(2):
                                    nc.tensor.matmul(
                                        ps_o[h][:, n * 512 : (n + 1) * 512],
                                        vp_c[:, t, h * 65 : (h + 1) * 65],
                                        pts[t][:, n * 512 : (n + 1) * 512],
                                        start=(t == 0),
                                        stop=(t == 3),
                                    )
                            if c == 0:
                                nc.vector.tensor_copy(oacc[h][:], ps_o[h][:])
                            else:
                                nc.vector.tensor_add(
                                    oacc[h][:], oacc[h][:], ps_o[h][:]
                                )
                            if c == N_CHUNKS - 1:
                                normalize_head(h)

                # ---- P4b: Y = O^T.T @ Wo in a 4-deep PSUM pool ----
                with (
                    tc.tile_pool(name="ypsum", bufs=4, space="PSUM") as yppool,
                    tc.tile_pool(name="yb", bufs=4) as ypool,
                ):
                    # kt=0 for a quad of mq tiles (needs only heads 0,1),
                    # then kt=1 per tile with the PSUM->SBUF copy emitted
                    # immediately after each stop, alternating ACT/DVE so
                    # the final drain isn't serialized on one engine.
                    for mq0 in range(0, NQ // 128, 4):
                        psy = {}
                        for mq in range(mq0, mq0 + 4):
                            psy[mq] = yppool.tile(
                                [128, NQ], f32, tag="ypsum", name=f"psy{mq}"
                            )
                            for n in range(2):
                                nc.tensor.matmul(
                                    psy[mq][:, n * 512 : (n + 1) * 512],
                                    ot[:, 0, mq * 128 : (mq + 1) * 128],
                                    wo_sb[:, 0, n * 512 : (n + 1) * 512],
                                    start=True,
                                    stop=False,
                                )
                        for mq in range(mq0, mq0 + 4):
                            for n in range(2):
                                nc.tensor.matmul(
                                    psy[mq][:, n * 512 : (n + 1) * 512],
                                    ot[:, 1, mq * 128 : (mq + 1) * 128],
                                    wo_sb[:, 1, n * 512 : (n + 1) * 512],
                                    start=False,
                                    stop=True,
                                )
                            yt = ypool.tile([128, NQ], bf16, tag="yt")
                            if mq % 2 == 0:
                                nc.scalar.copy(yt[:], psy[mq][:])
                            else:
                                nc.vector.tensor_copy(yt[:], psy[mq][:])
                            nc.sync.dma_start(
                                y[mq * 128 : (mq + 1) * 128, :], yt[:]
                            )

    nc.compile()
    _prog_cache[key] = nc
    return nc


def kernel(query, key, value, objectness_scores, Wq, bq, Wk, bk, Wv, bv, Wo, bo,
           _trace=False):
    import ml_dtypes
    from concourse.bass_utils import run_bass_kernel_spmd

    f = np.float32
    bft = ml_dtypes.bfloat16
    query = np.asarray(query, f)
    key_ = np.asarray(key, f)
    value = np.asarray(value, f)
    objs = np.asarray(objectness_scores, f)
    Wq = np.asarray(Wq, f); bq = np.asarray(bq, f)
    Wk = np.asarray(Wk, f); bk = np.asarray(bk, f)
    Wv = np.asarray(Wv, f); bv = np.asarray(bv, f)
    Wo = np.asarray(Wo, f); bo = np.asarray(bo, f)

    scale = np.float32(HD ** -0.5)
    has_bq = bool(np.any(bq)); has_bk = bool(np.any(bk)); has_bv = bool(np.any(bv))
    nc = _build(has_bq, has_bk, has_bv)

    xqt_b = [np.ascontiguousarray(query[b].T.astype(bft)) for b in range(B)]
    xkt_b = [np.ascontiguousarray(key_[b].T.astype(bft)) for b in range(B)]
    xvt_b = [np.ascontiguousarray(value[b].T.astype(bft)) for b in range(B)]
    objt_b = [np.ascontiguousarray(objs[b].reshape(-1, 128).T) for b in range(B)]

    in_maps = []
    for c in range(NCORES):
        b, g = divmod(c, NCORES // B)
        sl = slice(g * DH, (g + 1) * DH)
        m = {
            "xqt": xqt_b[b],
            "xkt": xkt_b[b],
            "xvt": xvt_b[b],
            "wq": np.ascontiguousarray((Wq[:, sl] * scale).astype(bft)),
            "wk": np.ascontiguousarray(Wk[:, sl].astype(bft)),
            "wv": np.ascontiguousarray(Wv[:, sl].astype(bft)),
            "wo": np.ascontiguousarray(Wo[sl, :]),
            "objt": objt_b[b],
        }
        if has_bq:
            m["bq"] = np.ascontiguousarray(bq[sl] * scale)
        if has_bk:
            m["bk"] = np.ascontiguousarray(bk[sl])
        if has_bv:
            m["bv"] = np.ascontiguousarray(bv[sl].astype(bft))
        in_maps.append(m)

    res = run_bass_kernel_spmd(
        nc, in_maps, core_ids=list(range(NCORES)), trace=_trace
    )
    out = np.zeros((B, NQ, DIM), np.float64)
    for c in range(NCORES):
        out[c // (NCORES // B)] += np.asarray(res.results[c]["y"]).astype(np.float64)
    out += bo.astype(np.float64)
    result = out.astype(np.float32)
    if _trace:
        return result, res
    return result
